# revision 19
# baseline (speedup 1.0000x reference)
"""Trainium2 Bass kernel for nn_Attention_Module (sparse_attention).

Computation per batch b (x_b: [C=256, T=4096] fp32):
    energy = x_b @ x_b^T                      # (256, 256), K=4096
    attn   = softmax(rowmax(energy) - energy) # == exp(mu - e)/Z, mu = rowmin
    out    = gamma * (attn @ x_b) + x_b

Strategy (8 cores, pure data-parallel, 4 batches/core):
  - x is loaded ONCE per batch (fp32, host-swizzled so every DMA run is
    16-32 KB contiguous per partition).  The second layout needed by
    matmul2 (c-on-partition) is built on-chip: fp32->fp16 cast on the
    vector engine, then one xbar DMA transpose per (batch, m).  The xbar
    out AP must be [128, E, 128] (last dim = in partition count); each
    128-col block of the input transposes cleanly into one plane, so the
    result has NO column permutation.
  - Attention is built as A''' = gamma*P/Z + I in natural [i, j] layout
    (gamma, 1/Z and the +x residual all folded in), transposed by one
    small xbar to At[j', 2m+jb, i']; matmul2 (A'''^T as fp16 weights)
    then produces the FINAL output directly and the epilogue is a pure
    fp32->fp16 copy split across vector/scalar.
  - matmul2 is ordered in same-weight runs of 4 so the weight reload is
    elided; matmul1 runs float32r (fp22 multiplies, fp32 accumulate).
  - Output stored fp16 (tolerance 2e-2); host upcasts.
"""

import numpy as np

B, C, T = 32, 256, 4096
NCORES = 8
NB = B // NCORES  # batches per core
P = 128
KT = T // P  # 32 k-tiles for the energy matmul
TC = T // 512  # 8 t-chunks for matmul2

_CACHE = {}


def _build_nc(variant=None):
    variant = variant or {}
    from contextlib import ExitStack

    import concourse.bacc as bacc
    import concourse.bass as bass
    import concourse.tile as tile
    from concourse import mybir

    f32 = mybir.dt.float32
    f32r = mybir.dt.float32r
    f16 = mybir.dt.float16
    ts = bass.ts

    nc = bacc.Bacc(
        "TRN2",
        target_bir_lowering=False,
        debug=False,
        enable_asserts=False,
        num_devices=NCORES,
    )

    # host-swizzled: xt[b, p, k*C + c] = x[b, c, k*128 + p]
    xt_h = nc.dram_tensor("xt", [NB, P, KT * C], f32r, kind="ExternalInput")
    # aux: [gamma, pad, pad, pad, identity-row(128) fp32]
    aux_h = nc.dram_tensor("aux", [P, 132], f32, kind="ExternalInput")
    o_h = nc.dram_tensor("o", [NB, P, 2 * T], f16, kind="ExternalOutput")

    with tile.TileContext(nc) as tc:
        with ExitStack() as ctx:
            singles = ctx.enter_context(tc.tile_pool(name="singles", bufs=1))
            xt_pool = ctx.enter_context(tc.tile_pool(name="xt", bufs=2))
            xq_pool = ctx.enter_context(tc.tile_pool(name="xq", bufs=1))
            y16_pool = ctx.enter_context(tc.tile_pool(name="y16", bufs=2))
            xn_pool = ctx.enter_context(tc.tile_pool(name="xn", bufs=2))
            out_pool = ctx.enter_context(tc.tile_pool(name="out", bufs=3))
            att_pool = ctx.enter_context(tc.tile_pool(name="att", bufs=2))
            sm_pool = ctx.enter_context(tc.tile_pool(name="sm", bufs=2))
            psum_e = ctx.enter_context(
                tc.tile_pool(name="psum_e", bufs=2, space="PSUM")
            )
            psum_o = ctx.enter_context(
                tc.tile_pool(name="psum_o", bufs=3, space="PSUM")
            )

            xt_ap = xt_h.ap()
            o_ap = o_h.ap()

            # aux on the ACT ring so it doesn't delay the first xt load
            aux = singles.tile([P, 132], f32)
            nc.scalar.dma_start(aux[:], aux_h.ap())
            gv = aux[:, 0:1]
            ident16 = singles.tile([P, P], f16)
            nc.gpsimd.tensor_copy(ident16[:], aux[:, 4:132])

            # b0 loads split finer so matmul1 starts ASAP:
            # spans (in k units of 128 rows): 4, 4, 8, 8, 8
            B0_SPANS = [(0, 4), (4, 4), (8, 8), (16, 8), (24, 8)]
            KH = KT // 2

            def issue_loads(b):
                if b == 0:
                    tls = []
                    for i, (k0, kn) in enumerate(B0_SPANS):
                        t_ = xq_pool.tile(
                            [P, kn, C], f32r, tag=f"xq{i}", name=f"xq{i}"
                        )
                        nc.sync.dma_start(
                            t_[:], xt_ap[b][:, k0 * C : (k0 + kn) * C]
                        )
                        tls.append((k0, kn, t_))
                    return tls
                xta = xt_pool.tile([P, KH, C], f32r, tag="xta", name="xta")
                xtb = xt_pool.tile([P, KH, C], f32r, tag="xtb", name="xtb")
                nc.sync.dma_start(xta[:], xt_ap[b][:, : KH * C])
                nc.sync.dma_start(xtb[:], xt_ap[b][:, KH * C :])
                return [(0, KH, xta), (KH, KH, xtb)]

            def src_at(tls, k):
                for k0, kn, t_ in tls:
                    if k0 <= k < k0 + kn:
                        return t_, k - k0
                raise AssertionError

            tiles = {0: issue_loads(0)}
            pending = None  # (b, At, xn) awaiting matmul2

            for b in range(NB):
                tls = tiles.pop(b)
                if b + 1 < NB:
                    tiles[b + 1] = issue_loads(b + 1)

                # ---- cast to fp16 in xbar-input layout (vector engine) ----
                # y16[p, m, k, c'] = x[b, m*128 + c', k*128 + p]
                y16 = y16_pool.tile([P, 2, KT, P], f16, tag="y16", name="y16")
                for k0, kn, t_ in tls:
                    for m in range(2):
                        nc.vector.tensor_copy(
                            y16[:, m, k0 : k0 + kn, :], t_[:, :, ts(m, P)]
                        )

                # ---- xbar transpose: per-plane square transposes ----
                # xn[p, m, k, t'] = y16[t', m, k, p] = x[b, m*128+p, k*128+t']
                xn = xn_pool.tile([P, 2, KT, P], f16, tag="xn", name="xn")
                nc.sync.dma_start_transpose(xn[:, 0], y16[:, 0])
                nc.sync.dma_start_transpose(xn[:, 1], y16[:, 1])

                # ---- matmul1: energy blocks (both m in ONE psum bank) ----
                P2 = sm_pool.tile([P, 2, C], f16, tag="P2", name="P2")
                pe = psum_e.tile([P, 2, C], mybir.dt.float32, name="pe")
                for m in range(2):
                    for k in range(KT):
                        t_, kk = src_at(tls, k)
                        nc.tensor.matmul(
                            pe[:, m, :],
                            lhsT=t_[:, kk, ts(m, P)],
                            rhs=t_[:, kk, :],
                            start=(k == 0),
                            stop=(k == KT - 1),
                        )

                # ---- softmax -> A''' = gamma*P/Z + I (natural layout) ----
                for m in range(2):
                    mu = sm_pool.tile([P, 1], f32, tag="mu")
                    Zs = sm_pool.tile([P, 1], f32, tag="Zs")
                    Zb = sm_pool.tile([P, 1], f16, tag="Zb")
                    rZ = sm_pool.tile([P, 1], f32, tag="rZ")
                    rZg = sm_pool.tile([P, 1], f32, tag="rZg")
                    Pm = sm_pool.tile([P, C], f16, tag="Pm")
                    nc.vector.tensor_reduce(
                        mu[:], pe[:, m, :], axis=mybir.AxisListType.X,
                        op=mybir.AluOpType.min,
                    )
                    nc.scalar.activation(
                        Pm[:],
                        pe[:, m, :],
                        mybir.ActivationFunctionType.Exp,
                        bias=mu[:],
                        scale=-1.0,
                        accum_out=Zs[:],
                    )
                    nc.vector.tensor_copy(Zb[:], Zs[:])
                    nc.vector.reciprocal(rZ[:], Zb[:])
                    nc.vector.tensor_scalar_mul(rZg[:], rZ[:], gv)
                    nc.scalar.mul(P2[:, m, :], Pm[:], rZg[:])
                    nc.gpsimd.tensor_add(
                        P2[:, m, ts(m, P)], P2[:, m, ts(m, P)], ident16[:]
                    )

                # At[j', 2m+jb, i'] = A'''[m*128 + i', jb*128 + j']
                At = att_pool.tile([P, 4, P], f16, tag="At", name="At")
                nc.scalar.dma_start_transpose(At[:], P2[:])

                # ---- matmul2 for the PREVIOUS batch (software pipeline) ----
                this = (b, At, xn)
                todo = [pending] if pending is not None else []
                if b == NB - 1:
                    todo.append(this)
                    pending = None
                else:
                    pending = this
                for pb, pAt, pxn in todo:
                    for m in range(2):
                        ots = [
                            out_pool.tile(
                                [P, 1024], f16, tag=f"ot{c}", name=f"ot{c}"
                            )
                            for c in range(4)
                        ]
                        for g in range(TC // 4):
                            pos = [
                                psum_o.tile(
                                    [P, 1024], mybir.dt.float32,
                                    name="po", tag="po",
                                )
                                for j in range(2)
                            ]
                            for k in range(2):
                                for j in range(4):
                                    t8 = 4 * g + j
                                    nc.tensor.matmul(
                                        pos[j // 2][:, ts(j % 2, 512)],
                                        lhsT=pAt[:, 2 * m + k, :],
                                        rhs=pxn[:, k, 4 * t8 : 4 * t8 + 4, :],
                                        start=(k == 0),
                                        stop=(k == 1),
                                    )
                            for jj in range(2):
                                c = 2 * g + jj
                                if c % 2 == 0:
                                    nc.vector.tensor_copy(
                                        ots[c][:], pos[jj][:]
                                    )
                                else:
                                    nc.scalar.copy(ots[c][:], pos[jj][:])
                        for c in range(4):
                            nc.sync.dma_start(
                                o_ap[pb][:, m * T :][:, ts(c, 1024)], ots[c][:]
                            )

    nc.compile()
    return nc


def _get_nc():
    if "nc" not in _CACHE:
        _CACHE["nc"] = _build_nc()
    return _CACHE["nc"]


def _make_aux(gamma_val):
    aux = np.zeros((P, 132), dtype=np.float32)
    aux[:, 0] = gamma_val
    aux[:, 4:132] = np.eye(P, dtype=np.float32)
    return aux


def kernel(x, gamma, _trace=False):
    import concourse.bass_utils as bass_utils

    x = np.ascontiguousarray(np.asarray(x, dtype=np.float32))
    gamma = np.asarray(gamma, dtype=np.float32).reshape(-1)

    nc = _get_nc()

    aux = _make_aux(gamma[0])
    in_maps = []
    for d in range(NCORES):
        xs = x[d * NB : (d + 1) * NB]
        # xt[b, p, k*C+c] = x[b, c, k*128+p]  (fat contiguous runs)
        xt = np.ascontiguousarray(
            xs.transpose(0, 2, 1)
            .reshape(NB, KT, P, C)
            .transpose(0, 2, 1, 3)
            .reshape(NB, P, KT * C)
        )
        in_maps.append({"xt": xt, "aux": aux})

    res = bass_utils.run_bass_kernel_spmd(
        nc, in_maps, core_ids=list(range(NCORES)), trace=_trace
    )
    # o[b, p, m*T + t] = out[b, m*128+p, t]
    out = np.concatenate(
        [
            r["o"].reshape(NB, P, 2, T).transpose(0, 2, 1, 3).reshape(NB, C, T)
            for r in res.results
        ],
        axis=0,
    ).astype(np.float32)
    if _trace:
        _CACHE["last_results"] = res
    return out
